# revision 12
# baseline (speedup 1.0000x reference)
"""Trainium2 Bass kernel for beam-search top-k masking (nn_Beam_57612691308621).

Strategy: shard the prompt dim P=32 across 8 NeuronCores (4 prompts each).
Each core, fully on-device:
  1. Streams its (32 rows x 128000) probs shard through SBUF in chunks,
     computing per-128-element segment maxima (the only full-data pass).
  2. Selects top-16 segments per partition (max8/max_index/match_replace),
     re-gathers those segments from HBM via indirect DMA, and extracts the
     exact per-beam top-16 prob values + vocab indices.
  3. Computes log-probs (ScalarE Ln) for the 16 candidates/beam only
     (log is monotone, so per-beam ordering by prob == ordering by logprob),
     then does the joint (beam, vocab) top-16 per prompt, the first-step
     override, EOS masking, grow-alive / grow-fin top-8, and gathers +
     writes the output sequence rows (token inserted at cur_pos) on device.
Host only shards inputs / concatenates outputs.
"""

import os
import sys

for _p in ("/opt/trn_rl_repo", "/root/.axon_site", "/root/.axon_site/_ro/trn_rl_repo",
           "/root/.axon_site/_ro/pypackages"):
    if os.path.isdir(_p) and _p not in sys.path:
        sys.path.append(_p)

import numpy as np

import concourse.bass as bass
import concourse.bacc as bacc
import concourse.mybir as mybir
from concourse import tile

dt = mybir.dt
AF = mybir.ActivationFunctionType
ALU = mybir.AluOpType
AX = mybir.AxisListType

N_CORES = 8
P_FULL, D, V, S = 32, 8, 128000, 2048
Pc = P_FULL // N_CORES          # prompts per core = 4
R = Pc * D                      # beam rows per core = 32
Q = 4                           # row quarters -> R*Q = 128 partitions
Vq = V // Q                     # 32000 elems per partition
W = 128                         # segment width
NSEG = Vq // W                  # 250 segments per partition
NCHUNK = 10
CW = Vq // NCHUNK               # 3200 elems per chunk per partition
SEG_PER_CHUNK = CW // W         # 25
K2 = 16                         # 2*D candidates
EOS = 2
INF = 1.0e7
NEG = -3.0e38


def build_core_program(nc):
    f32, i32, u32 = dt.float32, dt.int32, dt.uint32

    probs = nc.dram_tensor("probs", (R, V), f32, kind="ExternalInput")
    alive_seq = nc.dram_tensor("alive_seq", (R, S), i32, kind="ExternalInput")
    fin_seq = nc.dram_tensor("fin_seq", (R, S), i32, kind="ExternalInput")
    alive_lp = nc.dram_tensor("alive_lp", (Pc, D), f32, kind="ExternalInput")
    fin_lp = nc.dram_tensor("fin_lp", (Pc, D), f32, kind="ExternalInput")
    sp_in = nc.dram_tensor("sp", (Pc, 1), f32, kind="ExternalInput")
    isf_in = nc.dram_tensor("isf", (Pc, 1), f32, kind="ExternalInput")
    curpos_in = nc.dram_tensor("curpos", (R, 1), i32, kind="ExternalInput")

    att_out = nc.dram_tensor("att", (Pc, D), i32, kind="ExternalOutput")
    na_seq_out = nc.dram_tensor("na_seq", (R, S), i32, kind="ExternalOutput")
    na_lp_out = nc.dram_tensor("na_lp", (Pc, D), f32, kind="ExternalOutput")
    nf_seq_out = nc.dram_tensor("nf_seq", (R, S), i32, kind="ExternalOutput")
    nf_lp_out = nc.dram_tensor("nf_lp", (Pc, D), f32, kind="ExternalOutput")

    with tile.TileContext(nc) as tc:
        with (
            tc.tile_pool(name="stream", bufs=3) as stream_pool,
            tc.tile_pool(name="work", bufs=1) as wp,
            tc.tile_pool(name="stage", bufs=1) as sgp,
        ):
            # ---------------- constants (inline tables) ----------------
            p_idx = np.arange(128)
            c128_np = np.concatenate([
                np.repeat(p_idx[:, None], K2, axis=1),          # partition idx
                np.repeat(np.arange(K2)[None, :], 128, axis=0), # slot iota
                ((p_idx % Q) * Vq)[:, None],                    # quarter offset
            ], axis=1).astype(np.float32)                       # (128, 33)
            c128_dram = nc.inline_tensor(c128_np, name="c128")
            c128 = wp.tile([128, 2 * K2 + 1], f32)
            nc.sync.dma_start(c128[:], c128_dram.ap())
            pf16 = c128[:, 0:K2]
            islot_f = c128[:, K2:2 * K2]
            qoff_f = c128[:, 2 * K2:2 * K2 + 1]

            c32_np = np.concatenate([
                np.repeat(np.arange(64)[None, :], 32, axis=0),
                np.repeat(np.arange(S)[None, :], 32, axis=0),
            ], axis=1).astype(np.float32)                       # (32, 64+2048)
            c32_dram = nc.inline_tensor(c32_np, name="c32")
            c32 = wp.tile([32, 64 + S], f32)
            nc.sync.dma_start(c32[:], c32_dram.ap())
            iota64_f = c32[:, 0:64]
            iotaS = c32[:, 64:64 + S]

            c4_np = np.concatenate([
                np.repeat(np.arange(128)[None, :], Pc, axis=0),
                np.repeat((np.arange(Pc) * D)[:, None], D, axis=1),
                np.ones((Pc, D)),
            ], axis=1).astype(np.float32)                       # (4, 144)
            c4_dram = nc.inline_tensor(c4_np, name="c4")
            c4 = wp.tile([Pc, 128 + 2 * D], f32)
            nc.sync.dma_start(c4[:], c4_dram.ap())
            iota128_f = c4[:, 0:128]
            iotaK2_4 = c4[:, 0:K2]
            iotaD_4 = c4[:, 0:D]
            base8_f = c4[:, 128:128 + D]
            ones8 = c4[:, 128 + D:128 + 2 * D]

            # ---------------- small input loads ----------------
            alive32 = wp.tile([R, 1], f32)          # alive_lp flattened per beam row
            nc.sync.dma_start(alive32[:], alive_lp.ap().rearrange("p d -> (p d)").unsqueeze(1))
            alive8 = wp.tile([Pc, D], f32)
            nc.sync.dma_start(alive8[:], alive_lp.ap())
            fin8 = wp.tile([Pc, D], f32)
            nc.sync.dma_start(fin8[:], fin_lp.ap())
            sp1 = wp.tile([Pc, 1], f32)
            nc.sync.dma_start(sp1[:], sp_in.ap())
            isf1 = wp.tile([Pc, 1], f32)
            nc.sync.dma_start(isf1[:], isf_in.ap())
            curpos = wp.tile([R, 1], i32)
            nc.sync.dma_start(curpos[:], curpos_in.ap())
            curpos_f = wp.tile([R, 1], f32)
            nc.vector.tensor_copy(curpos_f[:], curpos[:])
            u8 = dt.uint8
            sp_u8 = wp.tile([Pc, 1], u8)
            nc.vector.tensor_copy(sp_u8[:], sp1[:])
            isf_u8 = wp.tile([Pc, 1], u8)
            nc.vector.tensor_copy(isf_u8[:], isf1[:])

            # ---------------- phase 1: stream probs, segment maxima ----------------
            probs_t = probs.ap().rearrange("r (q v) -> (r q) v", q=Q)  # (128, 32000)
            segmax = wp.tile([128, NSEG], f32)
            for c in range(NCHUNK):
                chunk = stream_pool.tile([128, CW], f32, tag="chunk")
                nc.sync.dma_start(chunk[:], probs_t[:, c * CW:(c + 1) * CW])
                nc.vector.reduce_max(
                    segmax[:, c * SEG_PER_CHUNK:(c + 1) * SEG_PER_CHUNK].unsqueeze(2),
                    chunk[:].rearrange("p (s w) -> p s w", w=W),
                    axis=AX.X,
                )

            # ---------------- phase 2: top-16 segments per partition ----------------
            segv = wp.tile([128, K2], f32)
            segidx = wp.tile([128, K2], u32)
            sm2 = wp.tile([128, NSEG], f32)
            nc.vector.max(out=segv[:, 0:8], in_=segmax[:])
            nc.vector.max_index(out=segidx[:, 0:8], in_max=segv[:, 0:8], in_values=segmax[:])
            nc.vector.match_replace(out=sm2[:], in_to_replace=segv[:, 0:8],
                                    in_values=segmax[:], imm_value=NEG)
            nc.vector.max(out=segv[:, 8:16], in_=sm2[:])
            nc.vector.max_index(out=segidx[:, 8:16], in_max=segv[:, 8:16], in_values=sm2[:])

            lseg_f = wp.tile([128, K2], f32)        # local seg id per slot, f32
            nc.vector.tensor_copy(lseg_f[:], segidx[:])
            gseg_f = wp.tile([128, K2], f32)        # global seg id = p*250 + local
            nc.vector.tensor_scalar(gseg_f[:], pf16, float(NSEG), None, op0=ALU.mult)
            nc.vector.tensor_add(gseg_f[:], gseg_f[:], lseg_f[:])
            gseg_i = wp.tile([128, K2], i32)
            nc.vector.tensor_copy(gseg_i[:], gseg_f[:])

            # ---------------- phase 3: drill-down gather + exact per-quarter top-16 ----
            gath = wp.tile([128, K2 * W], f32)
            probs_seg = probs.ap().rearrange("r (s w) -> (r s) w", w=W)
            for s in range(K2):
                nc.gpsimd.indirect_dma_start(
                    out=gath[:, s * W:(s + 1) * W],
                    out_offset=None,
                    in_=probs_seg,
                    in_offset=bass.IndirectOffsetOnAxis(ap=gseg_i[:, s:s + 1], axis=0),
                )
            qv = wp.tile([128, K2], f32)            # quarter top-16 values
            qpos = wp.tile([128, K2], u32)          # positions in gath (0..2047)
            g2 = wp.tile([128, K2 * W], f32)
            nc.vector.max(out=qv[:, 0:8], in_=gath[:])
            nc.vector.max_index(out=qpos[:, 0:8], in_max=qv[:, 0:8], in_values=gath[:])
            nc.vector.match_replace(out=g2[:], in_to_replace=qv[:, 0:8],
                                    in_values=gath[:], imm_value=NEG)
            nc.vector.max(out=qv[:, 8:16], in_=g2[:])
            nc.vector.max_index(out=qpos[:, 8:16], in_max=qv[:, 8:16], in_values=g2[:])

            slot_u = wp.tile([128, K2], u32)
            nc.vector.tensor_scalar(slot_u[:], qpos[:], 7, None, op0=ALU.logical_shift_right)
            off_u = wp.tile([128, K2], u32)
            nc.vector.tensor_scalar(off_u[:], qpos[:], W - 1, None, op0=ALU.bitwise_and)
            slot_f = wp.tile([128, K2], f32)
            nc.vector.tensor_copy(slot_f[:], slot_u[:])
            off_f = wp.tile([128, K2], f32)
            nc.vector.tensor_copy(off_f[:], off_u[:])

            # local seg id of each candidate via one-hot over the 16 slots
            eq3 = wp.tile([128, K2 * K2], f32)
            eq3v = eq3[:].rearrange("p (c s) -> p c s", s=K2)
            nc.vector.tensor_tensor(
                eq3v,
                slot_f[:].unsqueeze(2).broadcast_to([128, K2, K2]),
                islot_f.unsqueeze(1).broadcast_to([128, K2, K2]),
                op=ALU.is_equal,
            )
            nc.vector.tensor_tensor(
                eq3v, eq3v,
                lseg_f[:].unsqueeze(1).broadcast_to([128, K2, K2]),
                op=ALU.mult,
            )
            lcand_f = wp.tile([128, K2], f32)
            nc.vector.reduce_sum(lcand_f[:].unsqueeze(2), eq3v, axis=AX.X)

            # vocab index (within the beam row) of each candidate
            qvoc = wp.tile([128, K2], f32)
            nc.vector.tensor_scalar(qvoc[:], lcand_f[:], float(W), qoff_f,
                                    op0=ALU.mult, op1=ALU.add)
            nc.vector.tensor_add(qvoc[:], qvoc[:], off_f[:])

            # ---------------- phase 4: merge quarters -> per-beam-row top-16 ---------
            rowc = wp.tile([32, 128], f32)          # [vals(64) | vocab(64)]
            nc.sync.dma_start(
                rowc[:, 0:64].rearrange("r (q c) -> r q c", q=Q), qv[:])
            nc.sync.dma_start(
                rowc[:, 64:128].rearrange("r (q c) -> r q c", q=Q), qvoc[:])

            bv = wp.tile([32, K2], f32)             # per-beam top-16 prob values
            bpos = wp.tile([32, K2], u32)
            r2 = wp.tile([32, 64], f32)
            nc.vector.max(out=bv[:, 0:8], in_=rowc[:, 0:64])
            nc.vector.max_index(out=bpos[:, 0:8], in_max=bv[:, 0:8], in_values=rowc[:, 0:64])
            nc.vector.match_replace(out=r2[:], in_to_replace=bv[:, 0:8],
                                    in_values=rowc[:, 0:64], imm_value=NEG)
            nc.vector.max(out=bv[:, 8:16], in_=r2[:])
            nc.vector.max_index(out=bpos[:, 8:16], in_max=bv[:, 8:16], in_values=r2[:])

            bpos_f = wp.tile([32, K2], f32)
            nc.vector.tensor_copy(bpos_f[:], bpos[:])
            eqb = wp.tile([32, K2 * 64], f32)
            eqbv = eqb[:].rearrange("r (c s) -> r c s", s=64)
            nc.vector.tensor_tensor(
                eqbv,
                bpos_f[:].unsqueeze(2).broadcast_to([32, K2, 64]),
                iota64_f.unsqueeze(1).broadcast_to([32, K2, 64]),
                op=ALU.is_equal,
            )
            nc.vector.tensor_tensor(
                eqbv, eqbv,
                rowc[:, 64:128].unsqueeze(1).broadcast_to([32, K2, 64]),
                op=ALU.mult,
            )
            btok = wp.tile([32, K2], f32)           # vocab index per beam candidate
            nc.vector.reduce_sum(btok[:].unsqueeze(2), eqbv, axis=AX.X)

            # ---------------- phase 5: log + joint top-16 per prompt ----------------
            logp = wp.tile([32, K2], f32)
            nc.scalar.activation(logp[:], bv[:], AF.Ln)
            curr = wp.tile([32, K2], f32)
            nc.vector.tensor_scalar(curr[:], logp[:], alive32[:, 0:1], None, op0=ALU.add)

            currp = wp.tile([Pc, D * K2], f32)      # (4, 128) joint candidates
            nc.sync.dma_start(currp[:].rearrange("p (d c) -> p d c", d=D), curr[:])
            tokp = wp.tile([Pc, D * K2], f32)
            nc.sync.dma_start(tokp[:].rearrange("p (d c) -> p d c", d=D), btok[:])

            jv = wp.tile([Pc, K2], f32)             # joint top-16 logprob values
            jpos = wp.tile([Pc, K2], u32)
            j2 = wp.tile([Pc, D * K2], f32)
            nc.vector.max(out=jv[:, 0:8], in_=currp[:])
            nc.vector.max_index(out=jpos[:, 0:8], in_max=jv[:, 0:8], in_values=currp[:])
            nc.vector.match_replace(out=j2[:], in_to_replace=jv[:, 0:8],
                                    in_values=currp[:], imm_value=NEG)
            nc.vector.max(out=jv[:, 8:16], in_=j2[:])
            nc.vector.max_index(out=jpos[:, 8:16], in_max=jv[:, 8:16], in_values=j2[:])

            jpos_f = wp.tile([Pc, K2], f32)
            nc.vector.tensor_copy(jpos_f[:], jpos[:])
            beam_u = wp.tile([Pc, K2], u32)
            nc.vector.tensor_scalar(beam_u[:], jpos[:], 4, None, op0=ALU.logical_shift_right)
            beam_f = sgp.tile([Pc, K2], f32)         # parent beam of each topk entry
            nc.vector.tensor_copy(beam_f[:], beam_u[:])

            eqj = wp.tile([Pc, K2 * 128], f32)
            eqjv = eqj[:].rearrange("p (c s) -> p c s", s=128)
            nc.vector.tensor_tensor(
                eqjv,
                jpos_f[:].unsqueeze(2).broadcast_to([Pc, K2, 128]),
                iota128_f.unsqueeze(1).broadcast_to([Pc, K2, 128]),
                op=ALU.is_equal,
            )
            nc.vector.tensor_tensor(
                eqjv, eqjv,
                tokp[:].unsqueeze(1).broadcast_to([Pc, K2, 128]),
                op=ALU.mult,
            )
            jtok = wp.tile([Pc, K2], f32)           # token of each topk entry
            nc.vector.reduce_sum(jtok[:].unsqueeze(2), eqjv, axis=AX.X)

            # ---------------- phase 6: first-generation override --------------------
            isfb = isf_u8[:, 0:1].broadcast_to([Pc, K2])
            nc.vector.copy_predicated(jv[:], isfb, currp[:, 0:K2])
            nc.vector.copy_predicated(jtok[:], isfb, tokp[:, 0:K2])

            # ---------------- phase 7: grow_alive / grow_fin ------------------------
            fino = wp.tile([Pc, K2], f32)           # finished = tok == EOS
            nc.vector.tensor_scalar(fino[:], jtok[:], float(EOS), None, op0=ALU.is_equal)

            am = wp.tile([Pc, K2], f32)             # alive-masked = jv + fino*(-INF)
            nc.vector.tensor_scalar(am[:], fino[:], -INF, None, op0=ALU.mult)
            nc.vector.tensor_add(am[:], am[:], jv[:])

            nav = wp.tile([Pc, D], f32)
            nap = wp.tile([Pc, D], u32)
            nc.vector.max(out=nav[:], in_=am[:])
            nc.vector.max_index(out=nap[:], in_max=nav[:], in_values=am[:])
            nap_f = wp.tile([Pc, D], f32)
            nc.vector.tensor_copy(nap_f[:], nap[:])

            # gather beam/token of the 8 selected alive entries (one-hot over 16)
            eqa = wp.tile([Pc, D * K2], f32)
            eqav = eqa[:].rearrange("p (d c) -> p d c", c=K2)
            nc.vector.tensor_tensor(
                eqav,
                nap_f[:].unsqueeze(2).broadcast_to([Pc, D, K2]),
                iotaK2_4.unsqueeze(1).broadcast_to([Pc, D, K2]),
                op=ALU.is_equal,
            )
            tmpa = wp.tile([Pc, D * K2], f32)
            tmpav = tmpa[:].rearrange("p (d c) -> p d c", c=K2)
            nc.vector.tensor_tensor(
                tmpav, eqav, beam_f[:].unsqueeze(1).broadcast_to([Pc, D, K2]),
                op=ALU.mult)
            att_f = wp.tile([Pc, D], f32)
            nc.vector.reduce_sum(att_f[:].unsqueeze(2), tmpav, axis=AX.X)
            nc.vector.tensor_tensor(
                tmpav, eqav, jtok[:].unsqueeze(1).broadcast_to([Pc, D, K2]),
                op=ALU.mult)
            atok_f = wp.tile([Pc, D], f32)
            nc.vector.reduce_sum(atok_f[:].unsqueeze(2), tmpav, axis=AX.X)

            spb8 = sp_u8[:, 0:1].broadcast_to([Pc, D])

            # new_alive_lp = where(sp, alive_lp, nav)
            nalp = wp.tile([Pc, D], f32)
            nc.vector.tensor_copy(nalp[:], nav[:])
            nc.vector.copy_predicated(nalp[:], spb8, alive8[:])
            nc.sync.dma_start(na_lp_out.ap(), nalp[:])

            # attention_change_ids = where(sp, arange(D), att_f)
            nc.vector.copy_predicated(att_f[:], spb8, iotaD_4)
            att_i = wp.tile([Pc, D], i32)
            nc.vector.tensor_copy(att_i[:], att_f[:])
            nc.sync.dma_start(att_out.ap(), att_i[:])

            # alive seq source rows (within the core's 32): base + att (post-sp)
            arow_f = wp.tile([Pc, D], f32)
            nc.vector.tensor_add(arow_f[:], base8_f, att_f[:])

            # alive token-write flag = !sp
            wfa = wp.tile([Pc, D], f32)
            nc.vector.tensor_scalar(wfa[:], sp1[:, 0:1].broadcast_to([Pc, D]),
                                    -1.0, 1.0, op0=ALU.mult, op1=ALU.add)

            # ----- fin side -----
            notf = wp.tile([Pc, K2], f32)
            nc.vector.tensor_scalar(notf[:], fino[:], -1.0, 1.0, op0=ALU.mult, op1=ALU.add)
            fm = wp.tile([Pc, K2], f32)             # fin-masked = jv + (1-fino)*(-INF)
            nc.vector.tensor_scalar(fm[:], notf[:], -INF, None, op0=ALU.mult)
            nc.vector.tensor_add(fm[:], fm[:], jv[:])

            cat = wp.tile([Pc, D + K2], f32)        # [fin_lp(8) | fin_masked(16)]
            nc.vector.tensor_copy(cat[:, 0:D], fin8[:])
            nc.vector.tensor_copy(cat[:, D:D + K2], fm[:])

            nfv = wp.tile([Pc, D], f32)
            nfp = wp.tile([Pc, D], u32)
            nc.vector.max(out=nfv[:], in_=cat[:])
            nc.vector.max_index(out=nfp[:], in_max=nfv[:], in_values=cat[:])
            nfp_f = wp.tile([Pc, D], f32)
            nc.vector.tensor_copy(nfp_f[:], nfp[:])

            # new_fin_lp = where(sp, fin_lp, nfv)
            nflp = wp.tile([Pc, D], f32)
            nc.vector.tensor_copy(nflp[:], nfv[:])
            nc.vector.copy_predicated(nflp[:], spb8, fin8[:])
            nc.sync.dma_start(nf_lp_out.ap(), nflp[:])

            # old/new resolution
            isold = wp.tile([Pc, D], f32)           # nfp < 8 (or sp)
            nc.vector.tensor_scalar(isold[:], nfp_f[:], float(D), None, op0=ALU.is_lt)
            nc.vector.copy_predicated(isold[:], spb8, ones8)
            oldrow = wp.tile([Pc, D], f32)
            nc.vector.tensor_scalar(oldrow[:], nfp_f[:], float(D - 1), None, op0=ALU.min)
            nc.vector.copy_predicated(oldrow[:], spb8, iotaD_4)
            cand = wp.tile([Pc, D], f32)            # clamp(nfp-8, 0) -> topk entry id
            nc.vector.tensor_scalar(cand[:], nfp_f[:], float(D), 0.0,
                                    op0=ALU.subtract, op1=ALU.max)

            # gather beam/token for new-fin entries (one-hot over 16)
            eqf = wp.tile([Pc, D * K2], f32)
            eqfv = eqf[:].rearrange("p (d c) -> p d c", c=K2)
            nc.vector.tensor_tensor(
                eqfv,
                cand[:].unsqueeze(2).broadcast_to([Pc, D, K2]),
                iotaK2_4.unsqueeze(1).broadcast_to([Pc, D, K2]),
                op=ALU.is_equal,
            )
            nc.vector.tensor_tensor(
                tmpav, eqfv, beam_f[:].unsqueeze(1).broadcast_to([Pc, D, K2]),
                op=ALU.mult)
            fbeam_f = sgp.tile([Pc, D], f32)
            nc.vector.reduce_sum(fbeam_f[:].unsqueeze(2), tmpav, axis=AX.X)
            nc.vector.tensor_tensor(
                tmpav, eqfv, jtok[:].unsqueeze(1).broadcast_to([Pc, D, K2]),
                op=ALU.mult)
            ftok_f = wp.tile([Pc, D], f32)
            nc.vector.reduce_sum(ftok_f[:].unsqueeze(2), tmpav, axis=AX.X)

            frow_old_f = wp.tile([Pc, D], f32)
            nc.vector.tensor_add(frow_old_f[:], base8_f, oldrow[:])
            frow_new_f = wp.tile([Pc, D], f32)
            nc.vector.tensor_add(frow_new_f[:], base8_f, fbeam_f[:])

            # fin token-write flag = !sp & !isold ; new-source flag = !isold
            notold = wp.tile([Pc, D], f32)
            nc.vector.tensor_scalar(notold[:], isold[:], -1.0, 1.0, op0=ALU.mult, op1=ALU.add)
            wff = wp.tile([Pc, D], f32)
            nc.vector.tensor_tensor(wff[:], wfa[:], notold[:], op=ALU.mult)

            # pack per-row (32) controls, d-major:
            # [wfa, wff, notold, atok, ftok, arow, frow_old, frow_new]
            NF = 8
            pack = wp.tile([Pc, D * NF], f32)
            pack_v = pack[:].rearrange("p (d f) -> p d f", f=NF)
            fields = [wfa, wff, notold, atok_f, ftok_f,
                      arow_f, frow_old_f, frow_new_f]
            for f_i, fld in enumerate(fields):
                nc.vector.tensor_copy(pack_v[:, :, f_i:f_i + 1],
                                      fld[:].unsqueeze(2))
            unpk = wp.tile([R, NF], f32)
            nc.sync.dma_start(unpk[:], pack_v)
            atok_i = wp.tile([R, 1], i32)
            nc.vector.tensor_copy(atok_i[:], unpk[:, 3:4])
            ftok_i = wp.tile([R, 1], i32)
            nc.vector.tensor_copy(ftok_i[:], unpk[:, 4:5])
            arow_i = wp.tile([R, 1], i32)
            nc.vector.tensor_copy(arow_i[:], unpk[:, 5:6])
            frow_old_i = wp.tile([R, 1], i32)
            nc.vector.tensor_copy(frow_old_i[:], unpk[:, 6:7])
            frow_new_i = wp.tile([R, 1], i32)
            nc.vector.tensor_copy(frow_new_i[:], unpk[:, 7:8])

            # ---------------- seq gathers + token writes ----------------------------
            stage_a = sgp.tile([R, S], i32)
            nc.gpsimd.indirect_dma_start(
                out=stage_a[:], out_offset=None, in_=alive_seq.ap(),
                in_offset=bass.IndirectOffsetOnAxis(ap=arow_i[:], axis=0))
            stage_fo = sgp.tile([R, S], i32)
            nc.gpsimd.indirect_dma_start(
                out=stage_fo[:], out_offset=None, in_=fin_seq.ap(),
                in_offset=bass.IndirectOffsetOnAxis(ap=frow_old_i[:], axis=0))
            stage_fn = sgp.tile([R, S], i32)
            nc.gpsimd.indirect_dma_start(
                out=stage_fn[:], out_offset=None, in_=alive_seq.ap(),
                in_offset=bass.IndirectOffsetOnAxis(ap=frow_new_i[:], axis=0))

            # merge new-fin rows over old-fin staging
            notold_u8 = wp.tile([R, 1], dt.uint8)
            nc.vector.tensor_copy(notold_u8[:], unpk[:, 2:3])
            nc.vector.copy_predicated(
                stage_fo[:], notold_u8[:, 0:1].broadcast_to([R, S]), stage_fn[:])

            # token write masks: (col == cur_pos) * flag
            am_a = sgp.tile([R, S], dt.uint8)
            nc.vector.tensor_scalar(am_a[:], iotaS, curpos_f[:, 0:1], unpk[:, 0:1],
                                    op0=ALU.is_equal, op1=ALU.mult)
            nc.vector.copy_predicated(
                stage_a[:], am_a[:], atok_i[:, 0:1].broadcast_to([R, S]))
            am_f = sgp.tile([R, S], dt.uint8)
            nc.vector.tensor_scalar(am_f[:], iotaS, curpos_f[:, 0:1], unpk[:, 1:2],
                                    op0=ALU.is_equal, op1=ALU.mult)
            nc.vector.copy_predicated(
                stage_fo[:], am_f[:], ftok_i[:, 0:1].broadcast_to([R, S]))

            nc.sync.dma_start(na_seq_out.ap(), stage_a[:])
            nc.sync.dma_start(nf_seq_out.ap(), stage_fo[:])

    return nc


_CACHED_NC = None


def _get_nc():
    global _CACHED_NC
    if _CACHED_NC is None:
        nc = bacc.Bacc("TRN2", target_bir_lowering=False, debug=False)
        build_core_program(nc)
        nc.compile()
        _CACHED_NC = nc
    return _CACHED_NC


def make_in_maps(probs, alive_seq, fin_seq, alive_log_probs, fin_log_probs,
                 still_prompt, is_first, cur_pos):
    probs = np.asarray(probs, dtype=np.float32).reshape(P_FULL, D, V)
    alive_seq = np.asarray(alive_seq, dtype=np.int32)
    fin_seq = np.asarray(fin_seq, dtype=np.int32)
    alive_log_probs = np.asarray(alive_log_probs, dtype=np.float32)
    fin_log_probs = np.asarray(fin_log_probs, dtype=np.float32)
    still_prompt = np.asarray(still_prompt).astype(np.float32).reshape(P_FULL, 1)
    is_first = np.asarray(is_first).astype(np.float32).reshape(P_FULL, 1)
    cp = int(np.asarray(cur_pos))

    in_maps = []
    for c in range(N_CORES):
        s = slice(c * Pc, (c + 1) * Pc)
        in_maps.append({
            "probs": np.ascontiguousarray(probs[s].reshape(R, V)),
            "alive_seq": np.ascontiguousarray(alive_seq[s].reshape(R, S)),
            "fin_seq": np.ascontiguousarray(fin_seq[s].reshape(R, S)),
            "alive_lp": np.ascontiguousarray(alive_log_probs[s]),
            "fin_lp": np.ascontiguousarray(fin_log_probs[s]),
            "sp": np.ascontiguousarray(still_prompt[s]),
            "isf": np.ascontiguousarray(is_first[s]),
            "curpos": np.full((R, 1), cp, dtype=np.int32),
        })
    return in_maps


def assemble_outputs(results):
    att = np.concatenate([r["att"] for r in results], axis=0).astype(np.int32)
    na_seq = np.concatenate(
        [r["na_seq"].reshape(Pc, D, S) for r in results], axis=0).astype(np.int32)
    na_lp = np.concatenate([r["na_lp"] for r in results], axis=0).astype(np.float32)
    nf_seq = np.concatenate(
        [r["nf_seq"].reshape(Pc, D, S) for r in results], axis=0).astype(np.int32)
    nf_lp = np.concatenate([r["nf_lp"] for r in results], axis=0).astype(np.float32)
    return (att, na_seq, na_lp, nf_seq, nf_lp)


def kernel(probs, alive_seq, fin_seq, alive_log_probs, fin_log_probs,
           still_prompt, is_first, cur_pos, _trace=False, _trace_kwargs=None):
    from concourse.bass_utils import run_bass_kernel_spmd

    nc = _get_nc()
    in_maps = make_in_maps(probs, alive_seq, fin_seq, alive_log_probs,
                           fin_log_probs, still_prompt, is_first, cur_pos)
    res = run_bass_kernel_spmd(
        nc, in_maps, core_ids=list(range(N_CORES)), trace=_trace,
        **(_trace_kwargs or {}))
    out = assemble_outputs(res.results)
    if _trace:
        return out, res
    return out


# revision 14
# speedup vs baseline: 1.0706x; 1.0706x over previous
"""Trainium2 Bass kernel for beam-search top-k masking (nn_Beam_57612691308621).

Strategy: shard the prompt dim P=32 across 8 NeuronCores (4 prompts each).
Each core, fully on-device:
  1. Streams its (32 rows x 128000) probs shard through SBUF in chunks,
     computing per-128-element segment maxima (the only full-data pass),
     transposed per-chunk into a per-beam-row layout.
  2. Selects top-16 segments per beam row (max8/max_index/match_replace),
     re-gathers those segments from HBM via indirect DMA, and extracts the
     exact per-beam top-16 prob values + vocab indices.
  3. Computes log-probs (ScalarE Ln) for the 16 candidates/beam only
     (log is monotone, so per-beam ordering by prob == ordering by logprob),
     then does the joint (beam, vocab) top-16 per prompt, the first-step
     override, EOS masking, grow-alive / grow-fin top-8, gathers the output
     sequence rows on device, and writes the new token at cur_pos via a
     bounds-checked indirect scatter.
Host only shards inputs / concatenates outputs.
"""

import os
import sys

for _p in ("/opt/trn_rl_repo", "/root/.axon_site", "/root/.axon_site/_ro/trn_rl_repo",
           "/root/.axon_site/_ro/pypackages"):
    if os.path.isdir(_p) and _p not in sys.path:
        sys.path.append(_p)

import numpy as np

import concourse.bass as bass
import concourse.bacc as bacc
import concourse.mybir as mybir
from concourse import tile

dt = mybir.dt
AF = mybir.ActivationFunctionType
ALU = mybir.AluOpType
AX = mybir.AxisListType

N_CORES = 8
P_FULL, D, V, S = 32, 8, 128000, 2048
Pc = P_FULL // N_CORES          # prompts per core = 4
R = Pc * D                      # beam rows per core = 32
Q = 4                           # row quarters -> R*Q = 128 partitions
Vq = V // Q                     # 32000 elems per partition
W = 128                         # segment width
NSEG = Vq // W                  # 250 segments per partition
NSEG_ROW = V // W               # 1000 segments per beam row
NCHUNK = 10
CW = Vq // NCHUNK               # 3200 elems per chunk per partition
SEG_PER_CHUNK = CW // W         # 25
K2 = 16                         # 2*D candidates
EOS = 2
INF = 1.0e7
NEG = -3.0e38
OOB = 10 ** 9                   # scatter offset sentinel (dropped by bounds check)


def build_core_program(nc):
    f32, i32, u32, u8 = dt.float32, dt.int32, dt.uint32, dt.uint8

    probs = nc.dram_tensor("probs", (R, V), f32, kind="ExternalInput")
    seqs = nc.dram_tensor("seqs", (2 * R, S), i32, kind="ExternalInput")
    alive_lp = nc.dram_tensor("alive_lp", (Pc, D), f32, kind="ExternalInput")
    fin_lp = nc.dram_tensor("fin_lp", (Pc, D), f32, kind="ExternalInput")
    sp_in = nc.dram_tensor("sp", (Pc, 1), f32, kind="ExternalInput")
    isf_in = nc.dram_tensor("isf", (Pc, 1), f32, kind="ExternalInput")
    curpos_in = nc.dram_tensor("curpos", (R, 1), i32, kind="ExternalInput")

    att_out = nc.dram_tensor("att", (Pc, D), i32, kind="ExternalOutput")
    na_seq_out = nc.dram_tensor("na_seq", (R, S), i32, kind="ExternalOutput")
    na_lp_out = nc.dram_tensor("na_lp", (Pc, D), f32, kind="ExternalOutput")
    nf_seq_out = nc.dram_tensor("nf_seq", (R, S), i32, kind="ExternalOutput")
    nf_lp_out = nc.dram_tensor("nf_lp", (Pc, D), f32, kind="ExternalOutput")

    with tile.TileContext(nc) as tc:
        with (
            tc.tile_pool(name="stream", bufs=3) as stream_pool,
            tc.tile_pool(name="work", bufs=1) as wp,
            tc.tile_pool(name="stage", bufs=1) as sgp,
        ):
            # ------- phase 1 first: keep the SP HWDGE queue clear for streaming
            probs_t = probs.ap().rearrange("r (q v) -> (r q) v", q=Q)  # (128, 32000)
            segmax = wp.tile([128, NSEG], f32)
            segrow = wp.tile([R, NSEG_ROW], f32)    # per-beam-row segment maxima
            for c in range(NCHUNK):
                chunk = stream_pool.tile([128, CW], f32, tag="chunk")
                nc.sync.dma_start(chunk[:], probs_t[:, c * CW:(c + 1) * CW])
                seg_sl = segmax[:, c * SEG_PER_CHUNK:(c + 1) * SEG_PER_CHUNK]
                nc.vector.reduce_max(
                    seg_sl.unsqueeze(2),
                    chunk[:].rearrange("p (s w) -> p s w", w=W),
                    axis=AX.X,
                )
                # transpose this chunk's maxima into row-major layout (ACT queue)
                dst = segrow[:].rearrange("r (q s) -> r q s", q=Q)[
                    :, :, c * SEG_PER_CHUNK:(c + 1) * SEG_PER_CHUNK]
                nc.scalar.dma_start(dst, seg_sl)

            # ---------------- constants (inline tables, ACT queue) -----------
            c32_np = np.concatenate([
                np.arange(R)[:, None],                          # row id
                np.repeat(np.arange(K2)[None, :], R, axis=0),   # slot iota
            ], axis=1).astype(np.float32)                       # (32, 17)
            c32_dram = nc.inline_tensor(c32_np, name="c32")
            c32 = wp.tile([R, 1 + K2], f32)
            nc.scalar.dma_start(c32[:], c32_dram.ap())
            rowid_f = c32[:, 0:1]
            islot_f = c32[:, 1:1 + K2]

            c4_np = np.concatenate([
                np.repeat(np.arange(128)[None, :], Pc, axis=0),
                np.repeat((np.arange(Pc) * D)[:, None], D, axis=1),
                np.ones((Pc, D)),
            ], axis=1).astype(np.float32)                       # (4, 144)
            c4_dram = nc.inline_tensor(c4_np, name="c4")
            c4 = wp.tile([Pc, 128 + 2 * D], f32)
            nc.scalar.dma_start(c4[:], c4_dram.ap())
            iota128_f = c4[:, 0:128]
            iotaK2_4 = c4[:, 0:K2]
            iotaD_4 = c4[:, 0:D]
            base8_f = c4[:, 128:128 + D]
            ones8 = c4[:, 128 + D:128 + 2 * D]

            # ---------------- small input loads (ACT queue) ------------------
            alive32 = wp.tile([R, 1], f32)          # alive_lp per beam row
            nc.scalar.dma_start(alive32[:], alive_lp.ap().rearrange("p d -> (p d)").unsqueeze(1))
            alive8 = wp.tile([Pc, D], f32)
            nc.scalar.dma_start(alive8[:], alive_lp.ap())
            fin8 = wp.tile([Pc, D], f32)
            nc.scalar.dma_start(fin8[:], fin_lp.ap())
            sp1 = wp.tile([Pc, 1], f32)
            nc.scalar.dma_start(sp1[:], sp_in.ap())
            isf1 = wp.tile([Pc, 1], f32)
            nc.scalar.dma_start(isf1[:], isf_in.ap())
            curpos = wp.tile([R, 1], i32)
            nc.scalar.dma_start(curpos[:], curpos_in.ap())
            curpos_f = wp.tile([R, 1], f32)
            nc.vector.tensor_copy(curpos_f[:], curpos[:])
            sp_u8 = wp.tile([Pc, 1], u8)
            nc.vector.tensor_copy(sp_u8[:], sp1[:])
            isf_u8 = wp.tile([Pc, 1], u8)
            nc.vector.tensor_copy(isf_u8[:], isf1[:])

            # ---------------- phase 2: top-16 segments per beam row ----------
            segv = wp.tile([R, K2], f32)
            segidx = wp.tile([R, K2], u32)
            sm2 = wp.tile([R, NSEG_ROW], f32)
            nc.vector.max(out=segv[:, 0:8], in_=segrow[:])
            nc.vector.max_index(out=segidx[:, 0:8], in_max=segv[:, 0:8], in_values=segrow[:])
            nc.vector.match_replace(out=sm2[:], in_to_replace=segv[:, 0:8],
                                    in_values=segrow[:], imm_value=NEG)
            nc.vector.max(out=segv[:, 8:16], in_=sm2[:])
            nc.vector.max_index(out=segidx[:, 8:16], in_max=segv[:, 8:16], in_values=sm2[:])

            lseg_f = wp.tile([R, K2], f32)          # per-row local seg id (0..999)
            nc.vector.tensor_copy(lseg_f[:], segidx[:])
            gseg_f = wp.tile([R, K2], f32)          # global seg id = row*1000 + local
            nc.vector.tensor_scalar(gseg_f[:], rowid_f.broadcast_to([R, K2]),
                                    float(NSEG_ROW), None, op0=ALU.mult)
            nc.vector.tensor_add(gseg_f[:], gseg_f[:], lseg_f[:])
            gseg_i = wp.tile([R, K2], i32)
            nc.vector.tensor_copy(gseg_i[:], gseg_f[:])

            # ---------------- phase 3: drill-down gather + exact row top-16 --
            gath = wp.tile([R, K2 * W], f32)
            probs_seg = probs.ap().rearrange("r (s w) -> (r s) w", w=W)
            for s in range(K2):
                nc.gpsimd.indirect_dma_start(
                    out=gath[:, s * W:(s + 1) * W],
                    out_offset=None,
                    in_=probs_seg,
                    in_offset=bass.IndirectOffsetOnAxis(ap=gseg_i[:, s:s + 1], axis=0),
                )
            bv = wp.tile([R, K2], f32)              # per-beam-row top-16 prob values
            qpos = wp.tile([R, K2], u32)            # positions in gath (0..2047)
            g2 = wp.tile([R, K2 * W], f32)
            nc.vector.max(out=bv[:, 0:8], in_=gath[:])
            nc.vector.max_index(out=qpos[:, 0:8], in_max=bv[:, 0:8], in_values=gath[:])
            nc.vector.match_replace(out=g2[:], in_to_replace=bv[:, 0:8],
                                    in_values=gath[:], imm_value=NEG)
            nc.vector.max(out=bv[:, 8:16], in_=g2[:])
            nc.vector.max_index(out=qpos[:, 8:16], in_max=bv[:, 8:16], in_values=g2[:])

            slot_u = wp.tile([R, K2], u32)
            nc.vector.tensor_scalar(slot_u[:], qpos[:], 7, None, op0=ALU.logical_shift_right)
            off_u = wp.tile([R, K2], u32)
            nc.vector.tensor_scalar(off_u[:], qpos[:], W - 1, None, op0=ALU.bitwise_and)
            slot_f = wp.tile([R, K2], f32)
            nc.vector.tensor_copy(slot_f[:], slot_u[:])
            off_f = wp.tile([R, K2], f32)
            nc.vector.tensor_copy(off_f[:], off_u[:])

            # local seg id of each candidate via one-hot over the 16 slots
            eq3 = wp.tile([R, K2 * K2], f32)
            eq3v = eq3[:].rearrange("p (c s) -> p c s", s=K2)
            nc.vector.tensor_tensor(
                eq3v,
                slot_f[:].unsqueeze(2).broadcast_to([R, K2, K2]),
                islot_f.unsqueeze(1).broadcast_to([R, K2, K2]),
                op=ALU.is_equal,
            )
            nc.vector.tensor_tensor(
                eq3v, eq3v,
                lseg_f[:].unsqueeze(1).broadcast_to([R, K2, K2]),
                op=ALU.mult,
            )
            lcand_f = wp.tile([R, K2], f32)
            nc.vector.reduce_sum(lcand_f[:].unsqueeze(2), eq3v, axis=AX.X)

            btok = wp.tile([R, K2], f32)            # vocab index per beam candidate
            nc.vector.tensor_scalar(btok[:], lcand_f[:], float(W), None, op0=ALU.mult)
            nc.vector.tensor_add(btok[:], btok[:], off_f[:])

            # ---------------- phase 5: log + joint top-16 per prompt ---------
            logp = wp.tile([R, K2], f32)
            nc.scalar.activation(logp[:], bv[:], AF.Ln)
            curr = wp.tile([R, K2], f32)
            nc.vector.tensor_scalar(curr[:], logp[:], alive32[:, 0:1], None, op0=ALU.add)

            currp = wp.tile([Pc, D * K2], f32)      # (4, 128) joint candidates
            nc.sync.dma_start(currp[:].rearrange("p (d c) -> p d c", d=D), curr[:])
            tokp = wp.tile([Pc, D * K2], f32)
            nc.sync.dma_start(tokp[:].rearrange("p (d c) -> p d c", d=D), btok[:])

            jv = wp.tile([Pc, K2], f32)             # joint top-16 logprob values
            jpos = wp.tile([Pc, K2], u32)
            j2 = wp.tile([Pc, D * K2], f32)
            nc.vector.max(out=jv[:, 0:8], in_=currp[:])
            nc.vector.max_index(out=jpos[:, 0:8], in_max=jv[:, 0:8], in_values=currp[:])
            nc.vector.match_replace(out=j2[:], in_to_replace=jv[:, 0:8],
                                    in_values=currp[:], imm_value=NEG)
            nc.vector.max(out=jv[:, 8:16], in_=j2[:])
            nc.vector.max_index(out=jpos[:, 8:16], in_max=jv[:, 8:16], in_values=j2[:])

            jpos_f = wp.tile([Pc, K2], f32)
            nc.vector.tensor_copy(jpos_f[:], jpos[:])
            beam_u = wp.tile([Pc, K2], u32)
            nc.vector.tensor_scalar(beam_u[:], jpos[:], 4, None, op0=ALU.logical_shift_right)
            beam_f = wp.tile([Pc, K2], f32)         # parent beam of each topk entry
            nc.vector.tensor_copy(beam_f[:], beam_u[:])

            eqj = wp.tile([Pc, K2 * 128], f32)
            eqjv = eqj[:].rearrange("p (c s) -> p c s", s=128)
            nc.vector.tensor_tensor(
                eqjv,
                jpos_f[:].unsqueeze(2).broadcast_to([Pc, K2, 128]),
                iota128_f.unsqueeze(1).broadcast_to([Pc, K2, 128]),
                op=ALU.is_equal,
            )
            nc.vector.tensor_tensor(
                eqjv, eqjv,
                tokp[:].unsqueeze(1).broadcast_to([Pc, K2, 128]),
                op=ALU.mult,
            )
            jtok = wp.tile([Pc, K2], f32)           # token of each topk entry
            nc.vector.reduce_sum(jtok[:].unsqueeze(2), eqjv, axis=AX.X)

            # ---------------- phase 6: first-generation override -------------
            isfb = isf_u8[:, 0:1].broadcast_to([Pc, K2])
            nc.vector.copy_predicated(jv[:], isfb, currp[:, 0:K2])
            nc.vector.copy_predicated(jtok[:], isfb, tokp[:, 0:K2])

            # ---------------- phase 7: grow_alive / grow_fin -----------------
            fino = wp.tile([Pc, K2], f32)           # finished = tok == EOS
            nc.vector.tensor_scalar(fino[:], jtok[:], float(EOS), None, op0=ALU.is_equal)

            am = wp.tile([Pc, K2], f32)             # alive-masked = jv + fino*(-INF)
            nc.vector.tensor_scalar(am[:], fino[:], -INF, None, op0=ALU.mult)
            nc.vector.tensor_add(am[:], am[:], jv[:])

            nav = wp.tile([Pc, D], f32)
            nap = wp.tile([Pc, D], u32)
            nc.vector.max(out=nav[:], in_=am[:])
            nc.vector.max_index(out=nap[:], in_max=nav[:], in_values=am[:])
            nap_f = wp.tile([Pc, D], f32)
            nc.vector.tensor_copy(nap_f[:], nap[:])

            # gather beam/token of the 8 selected alive entries (one-hot over 16)
            eqa = wp.tile([Pc, D * K2], f32)
            eqav = eqa[:].rearrange("p (d c) -> p d c", c=K2)
            nc.vector.tensor_tensor(
                eqav,
                nap_f[:].unsqueeze(2).broadcast_to([Pc, D, K2]),
                iotaK2_4.unsqueeze(1).broadcast_to([Pc, D, K2]),
                op=ALU.is_equal,
            )
            tmpa = wp.tile([Pc, D * K2], f32)
            tmpav = tmpa[:].rearrange("p (d c) -> p d c", c=K2)
            nc.vector.tensor_tensor(
                tmpav, eqav, beam_f[:].unsqueeze(1).broadcast_to([Pc, D, K2]),
                op=ALU.mult)
            att_f = wp.tile([Pc, D], f32)
            nc.vector.reduce_sum(att_f[:].unsqueeze(2), tmpav, axis=AX.X)
            nc.vector.tensor_tensor(
                tmpav, eqav, jtok[:].unsqueeze(1).broadcast_to([Pc, D, K2]),
                op=ALU.mult)
            atok_f = wp.tile([Pc, D], f32)
            nc.vector.reduce_sum(atok_f[:].unsqueeze(2), tmpav, axis=AX.X)

            spb8 = sp_u8[:, 0:1].broadcast_to([Pc, D])

            # new_alive_lp = where(sp, alive_lp, nav)
            nalp = wp.tile([Pc, D], f32)
            nc.vector.tensor_copy(nalp[:], nav[:])
            nc.vector.copy_predicated(nalp[:], spb8, alive8[:])
            nc.sync.dma_start(na_lp_out.ap(), nalp[:])

            # attention_change_ids = where(sp, arange(D), att_f)
            nc.vector.copy_predicated(att_f[:], spb8, iotaD_4)
            att_i = wp.tile([Pc, D], i32)
            nc.vector.tensor_copy(att_i[:], att_f[:])
            nc.sync.dma_start(att_out.ap(), att_i[:])

            # alive seq source rows: base + att (post-sp)
            arow_f = wp.tile([Pc, D], f32)
            nc.vector.tensor_add(arow_f[:], base8_f, att_f[:])

            # alive token-write flag = !sp
            wfa = wp.tile([Pc, D], f32)
            nc.vector.tensor_scalar(wfa[:], sp1[:, 0:1].broadcast_to([Pc, D]),
                                    -1.0, 1.0, op0=ALU.mult, op1=ALU.add)

            # ----- fin side -----
            notf = wp.tile([Pc, K2], f32)
            nc.vector.tensor_scalar(notf[:], fino[:], -1.0, 1.0, op0=ALU.mult, op1=ALU.add)
            fm = wp.tile([Pc, K2], f32)             # fin-masked = jv + (1-fino)*(-INF)
            nc.vector.tensor_scalar(fm[:], notf[:], -INF, None, op0=ALU.mult)
            nc.vector.tensor_add(fm[:], fm[:], jv[:])

            cat = wp.tile([Pc, D + K2], f32)        # [fin_lp(8) | fin_masked(16)]
            nc.vector.tensor_copy(cat[:, 0:D], fin8[:])
            nc.vector.tensor_copy(cat[:, D:D + K2], fm[:])

            nfv = wp.tile([Pc, D], f32)
            nfp = wp.tile([Pc, D], u32)
            nc.vector.max(out=nfv[:], in_=cat[:])
            nc.vector.max_index(out=nfp[:], in_max=nfv[:], in_values=cat[:])
            nfp_f = wp.tile([Pc, D], f32)
            nc.vector.tensor_copy(nfp_f[:], nfp[:])

            # new_fin_lp = where(sp, fin_lp, nfv)
            nflp = wp.tile([Pc, D], f32)
            nc.vector.tensor_copy(nflp[:], nfv[:])
            nc.vector.copy_predicated(nflp[:], spb8, fin8[:])
            nc.sync.dma_start(nf_lp_out.ap(), nflp[:])

            # old/new resolution
            isold = wp.tile([Pc, D], f32)           # nfp < 8 (or sp)
            nc.vector.tensor_scalar(isold[:], nfp_f[:], float(D), None, op0=ALU.is_lt)
            nc.vector.copy_predicated(isold[:], spb8, ones8)
            isold_u8 = wp.tile([Pc, D], u8)
            nc.vector.tensor_copy(isold_u8[:], isold[:])
            oldrow = wp.tile([Pc, D], f32)
            nc.vector.tensor_scalar(oldrow[:], nfp_f[:], float(D - 1), None, op0=ALU.min)
            nc.vector.copy_predicated(oldrow[:], spb8, iotaD_4)
            cand = wp.tile([Pc, D], f32)            # clamp(nfp-8, 0) -> topk entry id
            nc.vector.tensor_scalar(cand[:], nfp_f[:], float(D), 0.0,
                                    op0=ALU.subtract, op1=ALU.max)

            # gather beam/token for new-fin entries (one-hot over 16)
            eqf = wp.tile([Pc, D * K2], f32)
            eqfv = eqf[:].rearrange("p (d c) -> p d c", c=K2)
            nc.vector.tensor_tensor(
                eqfv,
                cand[:].unsqueeze(2).broadcast_to([Pc, D, K2]),
                iotaK2_4.unsqueeze(1).broadcast_to([Pc, D, K2]),
                op=ALU.is_equal,
            )
            nc.vector.tensor_tensor(
                tmpav, eqfv, beam_f[:].unsqueeze(1).broadcast_to([Pc, D, K2]),
                op=ALU.mult)
            fbeam_f = wp.tile([Pc, D], f32)
            nc.vector.reduce_sum(fbeam_f[:].unsqueeze(2), tmpav, axis=AX.X)
            nc.vector.tensor_tensor(
                tmpav, eqfv, jtok[:].unsqueeze(1).broadcast_to([Pc, D, K2]),
                op=ALU.mult)
            ftok_f = wp.tile([Pc, D], f32)
            nc.vector.reduce_sum(ftok_f[:].unsqueeze(2), tmpav, axis=AX.X)

            # fin source row in the combined seqs tensor:
            # old -> R + base + oldrow ; new -> base + fbeam
            frow_f = wp.tile([Pc, D], f32)
            nc.vector.tensor_add(frow_f[:], base8_f, fbeam_f[:])
            frow_old_f = wp.tile([Pc, D], f32)
            nc.vector.tensor_scalar(frow_old_f[:], oldrow[:], 1.0, float(R),
                                    op0=ALU.mult, op1=ALU.add)
            nc.vector.tensor_add(frow_old_f[:], frow_old_f[:], base8_f)
            nc.vector.copy_predicated(frow_f[:], isold_u8[:], frow_old_f[:])

            # fin token-write flag = !sp & !isold
            notold = wp.tile([Pc, D], f32)
            nc.vector.tensor_scalar(notold[:], isold[:], -1.0, 1.0, op0=ALU.mult, op1=ALU.add)
            wff = wp.tile([Pc, D], f32)
            nc.vector.tensor_tensor(wff[:], wfa[:], notold[:], op=ALU.mult)

            # pack per-row (32) controls, d-major: [wfa, wff, atok, ftok, arow, frow]
            NF = 6
            pack = wp.tile([Pc, D * NF], f32)
            pack_v = pack[:].rearrange("p (d f) -> p d f", f=NF)
            fields = [wfa, wff, atok_f, ftok_f, arow_f, frow_f]
            for f_i, fld in enumerate(fields):
                nc.vector.tensor_copy(pack_v[:, :, f_i:f_i + 1],
                                      fld[:].unsqueeze(2))
            unpk = wp.tile([R, NF], f32)
            nc.sync.dma_start(unpk[:], pack_v)
            atok_i = wp.tile([R, 1], i32)
            nc.vector.tensor_copy(atok_i[:], unpk[:, 2:3])
            ftok_i = wp.tile([R, 1], i32)
            nc.vector.tensor_copy(ftok_i[:], unpk[:, 3:4])
            arow_i = wp.tile([R, 1], i32)
            nc.vector.tensor_copy(arow_i[:], unpk[:, 4:5])
            frow_i = wp.tile([R, 1], i32)
            nc.vector.tensor_copy(frow_i[:], unpk[:, 5:6])

            # scatter element offsets: row*S + cur_pos, or OOB when no write
            nwa_u8 = wp.tile([R, 1], u8)            # !write_alive
            nc.vector.tensor_scalar(nwa_u8[:], unpk[:, 0:1], -1.0, 1.0,
                                    op0=ALU.mult, op1=ALU.add)
            nwf_u8 = wp.tile([R, 1], u8)
            nc.vector.tensor_scalar(nwf_u8[:], unpk[:, 1:2], -1.0, 1.0,
                                    op0=ALU.mult, op1=ALU.add)
            big_f = wp.tile([R, 1], f32)
            nc.vector.memset(big_f[:], float(OOB))
            aoff_f = wp.tile([R, 1], f32)
            nc.vector.tensor_scalar(aoff_f[:], rowid_f, float(S), curpos_f[:, 0:1],
                                    op0=ALU.mult, op1=ALU.add)
            foff_f = wp.tile([R, 1], f32)
            nc.vector.tensor_copy(foff_f[:], aoff_f[:])
            nc.vector.copy_predicated(aoff_f[:], nwa_u8[:], big_f[:])
            nc.vector.copy_predicated(foff_f[:], nwf_u8[:], big_f[:])
            aoff_i = wp.tile([R, 1], i32)
            nc.vector.tensor_copy(aoff_i[:], aoff_f[:])
            foff_i = wp.tile([R, 1], i32)
            nc.vector.tensor_copy(foff_i[:], foff_f[:])

            # ---------------- seq gathers + outputs --------------------------
            stage_a = sgp.tile([R, S], i32)
            nc.gpsimd.indirect_dma_start(
                out=stage_a[:], out_offset=None, in_=seqs.ap(),
                in_offset=bass.IndirectOffsetOnAxis(ap=arow_i[:], axis=0))
            stage_f = sgp.tile([R, S], i32)
            nc.gpsimd.indirect_dma_start(
                out=stage_f[:], out_offset=None, in_=seqs.ap(),
                in_offset=bass.IndirectOffsetOnAxis(ap=frow_i[:], axis=0))

            nc.sync.dma_start(na_seq_out.ap(), stage_a[:])
            nc.sync.dma_start(nf_seq_out.ap(), stage_f[:])

            # token writes at cur_pos via bounds-checked scatter (OOB dropped)
            nc.gpsimd.indirect_dma_start(
                out=na_seq_out.ap().rearrange("r s -> (r s)").unsqueeze(1),
                out_offset=bass.IndirectOffsetOnAxis(ap=aoff_i[:], axis=0),
                in_=atok_i[:],
                in_offset=None,
                bounds_check=R * S - 1,
                oob_is_err=False,
            )
            nc.gpsimd.indirect_dma_start(
                out=nf_seq_out.ap().rearrange("r s -> (r s)").unsqueeze(1),
                out_offset=bass.IndirectOffsetOnAxis(ap=foff_i[:], axis=0),
                in_=ftok_i[:],
                in_offset=None,
                bounds_check=R * S - 1,
                oob_is_err=False,
            )

    return nc


_CACHED_NC = None


def _get_nc():
    global _CACHED_NC
    if _CACHED_NC is None:
        nc = bacc.Bacc("TRN2", target_bir_lowering=False, debug=False)
        build_core_program(nc)
        nc.compile()
        _CACHED_NC = nc
    return _CACHED_NC


def make_in_maps(probs, alive_seq, fin_seq, alive_log_probs, fin_log_probs,
                 still_prompt, is_first, cur_pos):
    probs = np.asarray(probs, dtype=np.float32).reshape(P_FULL, D, V)
    alive_seq = np.asarray(alive_seq, dtype=np.int32)
    fin_seq = np.asarray(fin_seq, dtype=np.int32)
    alive_log_probs = np.asarray(alive_log_probs, dtype=np.float32)
    fin_log_probs = np.asarray(fin_log_probs, dtype=np.float32)
    still_prompt = np.asarray(still_prompt).astype(np.float32).reshape(P_FULL, 1)
    is_first = np.asarray(is_first).astype(np.float32).reshape(P_FULL, 1)
    cp = int(np.asarray(cur_pos))

    in_maps = []
    for c in range(N_CORES):
        s = slice(c * Pc, (c + 1) * Pc)
        seqs = np.concatenate([alive_seq[s].reshape(R, S),
                               fin_seq[s].reshape(R, S)], axis=0)
        in_maps.append({
            "probs": np.ascontiguousarray(probs[s].reshape(R, V)),
            "seqs": np.ascontiguousarray(seqs),
            "alive_lp": np.ascontiguousarray(alive_log_probs[s]),
            "fin_lp": np.ascontiguousarray(fin_log_probs[s]),
            "sp": np.ascontiguousarray(still_prompt[s]),
            "isf": np.ascontiguousarray(is_first[s]),
            "curpos": np.full((R, 1), cp, dtype=np.int32),
        })
    return in_maps


def assemble_outputs(results):
    att = np.concatenate([r["att"] for r in results], axis=0).astype(np.int32)
    na_seq = np.concatenate(
        [r["na_seq"].reshape(Pc, D, S) for r in results], axis=0).astype(np.int32)
    na_lp = np.concatenate([r["na_lp"] for r in results], axis=0).astype(np.float32)
    nf_seq = np.concatenate(
        [r["nf_seq"].reshape(Pc, D, S) for r in results], axis=0).astype(np.int32)
    nf_lp = np.concatenate([r["nf_lp"] for r in results], axis=0).astype(np.float32)
    return (att, na_seq, na_lp, nf_seq, nf_lp)


def kernel(probs, alive_seq, fin_seq, alive_log_probs, fin_log_probs,
           still_prompt, is_first, cur_pos, _trace=False, _trace_kwargs=None):
    from concourse.bass_utils import run_bass_kernel_spmd

    nc = _get_nc()
    in_maps = make_in_maps(probs, alive_seq, fin_seq, alive_log_probs,
                           fin_log_probs, still_prompt, is_first, cur_pos)
    res = run_bass_kernel_spmd(
        nc, in_maps, core_ids=list(range(N_CORES)), trace=_trace,
        **(_trace_kwargs or {}))
    out = assemble_outputs(res.results)
    if _trace:
        return out, res
    return out


# revision 18
# speedup vs baseline: 1.1095x; 1.0362x over previous
"""Trainium2 Bass kernel for beam-search top-k masking (nn_Beam_57612691308621).

Strategy: shard the prompt dim P=32 across 8 NeuronCores (4 prompts each).
Each core, fully on-device:
  1. Streams its (32 rows x 128000) probs shard through SBUF in chunks,
     computing per-128-element segment maxima (the only full-data pass),
     transposed per-chunk into a per-beam-row layout.
  2. Selects top-16 segments per beam row (max8/max_index/match_replace),
     re-gathers those segments from HBM via indirect DMA, and extracts the
     exact per-beam top-16 prob values + vocab indices.
  3. Computes log-probs (ScalarE Ln) for the 16 candidates/beam only
     (log is monotone, so per-beam ordering by prob == ordering by logprob),
     then does the joint (beam, vocab) top-16 per prompt, the first-step
     override, EOS masking, grow-alive / grow-fin top-8, gathers the output
     sequence rows on device, and writes the new token at cur_pos via a
     bounds-checked indirect scatter.
Host only shards inputs / concatenates outputs.
"""

import os
import sys

for _p in ("/opt/trn_rl_repo", "/root/.axon_site", "/root/.axon_site/_ro/trn_rl_repo",
           "/root/.axon_site/_ro/pypackages"):
    if os.path.isdir(_p) and _p not in sys.path:
        sys.path.append(_p)

import numpy as np

import concourse.bass as bass
import concourse.bacc as bacc
import concourse.mybir as mybir
from concourse import tile

dt = mybir.dt
AF = mybir.ActivationFunctionType
ALU = mybir.AluOpType
AX = mybir.AxisListType

N_CORES = 8
P_FULL, D, V, S = 32, 8, 128000, 2048
Pc = P_FULL // N_CORES          # prompts per core = 4
R = Pc * D                      # beam rows per core = 32
Q = 4                           # row quarters -> R*Q = 128 partitions
Vq = V // Q                     # 32000 elems per partition
W = 128                         # segment width
NSEG = Vq // W                  # 250 segments per partition
NSEG_ROW = V // W               # 1000 segments per beam row
NCHUNK = 10
CW = Vq // NCHUNK               # 3200 elems per chunk per partition
SEG_PER_CHUNK = CW // W         # 25
K2 = 16                         # 2*D candidates
EOS = 2
INF = 1.0e7
NEG = -3.0e38
OOB = 10 ** 9                   # scatter offset sentinel (dropped by bounds check)


def build_core_program(nc, cur_pos):
    f32, i32, u32, u8 = dt.float32, dt.int32, dt.uint32, dt.uint8

    probs = nc.dram_tensor("probs", (R, V), f32, kind="ExternalInput")
    seqs = nc.dram_tensor("seqs", (2 * R, S), i32, kind="ExternalInput")
    alive_lp = nc.dram_tensor("alive_lp", (Pc, D), f32, kind="ExternalInput")
    fin_lp = nc.dram_tensor("fin_lp", (Pc, D), f32, kind="ExternalInput")
    sp_in = nc.dram_tensor("sp", (Pc, 1), f32, kind="ExternalInput")
    isf_in = nc.dram_tensor("isf", (Pc, 1), f32, kind="ExternalInput")

    att_out = nc.dram_tensor("att", (Pc, D), i32, kind="ExternalOutput")
    na_seq_out = nc.dram_tensor("na_seq", (R, S), i32, kind="ExternalOutput")
    na_lp_out = nc.dram_tensor("na_lp", (Pc, D), f32, kind="ExternalOutput")
    nf_seq_out = nc.dram_tensor("nf_seq", (R, S), i32, kind="ExternalOutput")
    nf_lp_out = nc.dram_tensor("nf_lp", (Pc, D), f32, kind="ExternalOutput")

    with tile.TileContext(nc) as tc:
        with (
            tc.tile_pool(name="stream", bufs=3) as stream_pool,
            tc.tile_pool(name="work", bufs=1) as wp,
            tc.tile_pool(name="stage", bufs=1) as sgp,
        ):
            # ------- phase 1 first: keep the SP HWDGE queue clear for streaming
            probs_t = probs.ap().rearrange("r (q v) -> (r q) v", q=Q)  # (128, 32000)
            segmax = wp.tile([128, NSEG], f32)
            segrow = wp.tile([R, NSEG_ROW], f32)    # per-beam-row segment maxima
            for c in range(NCHUNK):
                chunk = stream_pool.tile([128, CW], f32, tag="chunk")
                nc.sync.dma_start(chunk[:], probs_t[:, c * CW:(c + 1) * CW])
                seg_sl = segmax[:, c * SEG_PER_CHUNK:(c + 1) * SEG_PER_CHUNK]
                nc.vector.reduce_max(
                    seg_sl.unsqueeze(2),
                    chunk[:].rearrange("p (s w) -> p s w", w=W),
                    axis=AX.X,
                )
                # transpose this chunk's maxima into row-major layout (ACT queue)
                dst = segrow[:].rearrange("r (q s) -> r q s", q=Q)[
                    :, :, c * SEG_PER_CHUNK:(c + 1) * SEG_PER_CHUNK]
                nc.scalar.dma_start(dst, seg_sl)

            # ---------------- constants (inline tables, ACT queue) -----------
            c32_np = np.concatenate([
                np.arange(R)[:, None],                          # row id
                (np.arange(R) * NSEG_ROW)[:, None],             # row seg base
                np.repeat(np.arange(K2)[None, :], R, axis=0),   # slot iota
            ], axis=1).astype(np.float32)                       # (32, 18)
            c32_dram = nc.inline_tensor(c32_np, name="c32")
            c32 = wp.tile([R, 2 + K2], f32)
            nc.scalar.dma_start(c32[:], c32_dram.ap())
            rowid_f = c32[:, 0:1]
            rowbase_f = c32[:, 1:2]
            islot_f = c32[:, 2:2 + K2]

            c4_np = np.concatenate([
                np.repeat(np.arange(128)[None, :], Pc, axis=0),
                np.repeat((np.arange(Pc) * D)[:, None], D, axis=1),
                np.ones((Pc, D)),
            ], axis=1).astype(np.float32)                       # (4, 144)
            c4_dram = nc.inline_tensor(c4_np, name="c4")
            c4 = wp.tile([Pc, 128 + 2 * D], f32)
            nc.scalar.dma_start(c4[:], c4_dram.ap())
            iota128_f = c4[:, 0:128]
            iotaK2_4 = c4[:, 0:K2]
            iotaD_4 = c4[:, 0:D]
            base8_f = c4[:, 128:128 + D]
            ones8 = c4[:, 128 + D:128 + 2 * D]

            # ---------------- small input loads (ACT queue) ------------------
            alive32 = wp.tile([R, 1], f32)          # alive_lp per beam row
            nc.scalar.dma_start(alive32[:], alive_lp.ap().rearrange("p d -> (p d)").unsqueeze(1))
            alive8 = wp.tile([Pc, D], f32)
            nc.scalar.dma_start(alive8[:], alive_lp.ap())
            fin8 = wp.tile([Pc, D], f32)
            nc.scalar.dma_start(fin8[:], fin_lp.ap())
            sp1 = wp.tile([Pc, 1], f32)
            nc.scalar.dma_start(sp1[:], sp_in.ap())
            isf1 = wp.tile([Pc, 1], f32)
            nc.scalar.dma_start(isf1[:], isf_in.ap())
            sp_u8 = wp.tile([Pc, 1], u8)
            nc.vector.tensor_copy(sp_u8[:], sp1[:])
            isf_u8 = wp.tile([Pc, 1], u8)
            nc.vector.tensor_copy(isf_u8[:], isf1[:])

            # ---------------- phase 2: top-16 segments per beam row ----------
            segv = wp.tile([R, K2], f32)
            segidx = wp.tile([R, K2], u32)
            sm2 = wp.tile([R, NSEG_ROW], f32)
            nc.vector.max(out=segv[:, 0:8], in_=segrow[:])
            nc.vector.max_index(out=segidx[:, 0:8], in_max=segv[:, 0:8], in_values=segrow[:])
            nc.vector.match_replace(out=sm2[:], in_to_replace=segv[:, 0:8],
                                    in_values=segrow[:], imm_value=NEG)
            nc.vector.max(out=segv[:, 8:16], in_=sm2[:])
            nc.vector.max_index(out=segidx[:, 8:16], in_max=segv[:, 8:16], in_values=sm2[:])

            lseg_f = wp.tile([R, K2], f32)          # per-row local seg id (0..999)
            nc.vector.tensor_copy(lseg_f[:], segidx[:])
            gseg_f = wp.tile([R, K2], f32)          # global seg id = row*1000 + local
            nc.vector.tensor_scalar(gseg_f[:], lseg_f[:], rowbase_f[:, 0:1], None,
                                    op0=ALU.add)
            gseg_i = wp.tile([R, K2], i32)
            nc.vector.tensor_copy(gseg_i[:], gseg_f[:])

            # ---------------- phase 3: drill-down gather + exact row top-16 --
            # distribute the 512 segment fetches over all 128 partitions:
            # partition (r, g) on call c fetches the row's slot s = c*4+g
            gsegP = wp.tile([R, K2], i32)           # permuted: [g*4+c] = gseg[c*4+g]
            nc.vector.tensor_copy(
                gsegP[:].rearrange("r (g c) -> r g c", g=Q),
                gseg_i[:].rearrange("r (c g) -> r g c", g=Q))
            gsegT = wp.tile([4 * R, Q], i32)
            nc.scalar.dma_start(gsegT[:], gsegP[:])
            gathQ = wp.tile([4 * R, Q * W], f32)
            probs_seg = probs.ap().rearrange("r (s w) -> (r s) w", w=W)
            for c in range(Q):
                nc.gpsimd.indirect_dma_start(
                    out=gathQ[:, c * W:(c + 1) * W],
                    out_offset=None,
                    in_=probs_seg,
                    in_offset=bass.IndirectOffsetOnAxis(ap=gsegT[:, c:c + 1], axis=0),
                )
            gath = wp.tile([R, K2 * W], f32)        # slotpos (g*4+c) -> slot c*4+g
            nc.sync.dma_start(
                gath[:].rearrange("r (g j) -> r g j", g=Q), gathQ[:])
            bv = wp.tile([R, K2], f32)              # per-beam-row top-16 prob values
            qpos = wp.tile([R, K2], u32)            # positions in gath (0..2047)
            g2 = wp.tile([R, K2 * W], f32)
            nc.vector.max(out=bv[:, 0:8], in_=gath[:])
            nc.vector.max_index(out=qpos[:, 0:8], in_max=bv[:, 0:8], in_values=gath[:])
            nc.vector.match_replace(out=g2[:], in_to_replace=bv[:, 0:8],
                                    in_values=gath[:], imm_value=NEG)
            nc.vector.max(out=bv[:, 8:16], in_=g2[:])
            nc.vector.max_index(out=qpos[:, 8:16], in_max=bv[:, 8:16], in_values=g2[:])

            slot_u = wp.tile([R, K2], u32)
            nc.vector.tensor_scalar(slot_u[:], qpos[:], 7, None, op0=ALU.logical_shift_right)
            off_u = wp.tile([R, K2], u32)
            nc.vector.tensor_scalar(off_u[:], qpos[:], W - 1, None, op0=ALU.bitwise_and)
            slot_f = wp.tile([R, K2], f32)
            nc.vector.tensor_copy(slot_f[:], slot_u[:])
            off_f = wp.tile([R, K2], f32)
            nc.vector.tensor_copy(off_f[:], off_u[:])

            # local seg id keyed by gath slot position (permuted c/g)
            lperm_f = wp.tile([R, K2], f32)
            nc.vector.tensor_copy(
                lperm_f[:].rearrange("r (g c) -> r g c", g=Q),
                lseg_f[:].rearrange("r (c g) -> r g c", g=Q))
            # one-hot over the 16 slot positions
            eq3 = wp.tile([R, K2 * K2], f32)
            eq3v = eq3[:].rearrange("p (c s) -> p c s", s=K2)
            nc.vector.tensor_tensor(
                eq3v,
                slot_f[:].unsqueeze(2).broadcast_to([R, K2, K2]),
                islot_f.unsqueeze(1).broadcast_to([R, K2, K2]),
                op=ALU.is_equal,
            )
            nc.vector.tensor_tensor(
                eq3v, eq3v,
                lperm_f[:].unsqueeze(1).broadcast_to([R, K2, K2]),
                op=ALU.mult,
            )
            lcand_f = wp.tile([R, K2], f32)
            nc.vector.reduce_sum(lcand_f[:].unsqueeze(2), eq3v, axis=AX.X)

            btok = wp.tile([R, K2], f32)            # vocab index per beam candidate
            nc.vector.tensor_scalar(btok[:], lcand_f[:], float(W), None, op0=ALU.mult)
            nc.vector.tensor_add(btok[:], btok[:], off_f[:])

            # ---------------- phase 5: log + joint top-16 per prompt ---------
            logp = wp.tile([R, K2], f32)
            nc.scalar.activation(logp[:], bv[:], AF.Ln)
            curr = wp.tile([R, K2], f32)
            nc.vector.tensor_scalar(curr[:], logp[:], alive32[:, 0:1], None, op0=ALU.add)

            currp = wp.tile([Pc, D * K2], f32)      # (4, 128) joint candidates
            nc.sync.dma_start(currp[:].rearrange("p (d c) -> p d c", d=D), curr[:])
            tokp = wp.tile([Pc, D * K2], f32)
            nc.sync.dma_start(tokp[:].rearrange("p (d c) -> p d c", d=D), btok[:])

            jv = wp.tile([Pc, K2], f32)             # joint top-16 logprob values
            jpos = wp.tile([Pc, K2], u32)
            j2 = wp.tile([Pc, D * K2], f32)
            nc.vector.max(out=jv[:, 0:8], in_=currp[:])
            nc.vector.max_index(out=jpos[:, 0:8], in_max=jv[:, 0:8], in_values=currp[:])
            nc.vector.match_replace(out=j2[:], in_to_replace=jv[:, 0:8],
                                    in_values=currp[:], imm_value=NEG)
            nc.vector.max(out=jv[:, 8:16], in_=j2[:])
            nc.vector.max_index(out=jpos[:, 8:16], in_max=jv[:, 8:16], in_values=j2[:])

            jpos_f = wp.tile([Pc, K2], f32)
            nc.vector.tensor_copy(jpos_f[:], jpos[:])
            beam_u = wp.tile([Pc, K2], u32)
            nc.vector.tensor_scalar(beam_u[:], jpos[:], 4, None, op0=ALU.logical_shift_right)
            beam_f = wp.tile([Pc, K2], f32)         # parent beam of each topk entry
            nc.vector.tensor_copy(beam_f[:], beam_u[:])

            eqj = wp.tile([Pc, K2 * 128], f32)
            eqjv = eqj[:].rearrange("p (c s) -> p c s", s=128)
            nc.vector.tensor_tensor(
                eqjv,
                jpos_f[:].unsqueeze(2).broadcast_to([Pc, K2, 128]),
                iota128_f.unsqueeze(1).broadcast_to([Pc, K2, 128]),
                op=ALU.is_equal,
            )
            nc.vector.tensor_tensor(
                eqjv, eqjv,
                tokp[:].unsqueeze(1).broadcast_to([Pc, K2, 128]),
                op=ALU.mult,
            )
            jtok = wp.tile([Pc, K2], f32)           # token of each topk entry
            nc.vector.reduce_sum(jtok[:].unsqueeze(2), eqjv, axis=AX.X)

            # ---------------- phase 6: first-generation override -------------
            isfb = isf_u8[:, 0:1].broadcast_to([Pc, K2])
            nc.vector.copy_predicated(jv[:], isfb, currp[:, 0:K2])
            nc.vector.copy_predicated(jtok[:], isfb, tokp[:, 0:K2])

            # ---------------- phase 7: grow_alive / grow_fin -----------------
            fino = wp.tile([Pc, K2], f32)           # finished = tok == EOS
            nc.vector.tensor_scalar(fino[:], jtok[:], float(EOS), None, op0=ALU.is_equal)

            am = wp.tile([Pc, K2], f32)             # alive-masked = jv + fino*(-INF)
            nc.vector.tensor_scalar(am[:], fino[:], -INF, None, op0=ALU.mult)
            nc.vector.tensor_add(am[:], am[:], jv[:])

            nav = wp.tile([Pc, D], f32)
            nap = wp.tile([Pc, D], u32)
            nc.vector.max(out=nav[:], in_=am[:])
            nc.vector.max_index(out=nap[:], in_max=nav[:], in_values=am[:])
            nap_f = wp.tile([Pc, D], f32)
            nc.vector.tensor_copy(nap_f[:], nap[:])

            # gather beam/token of the 8 selected alive entries (one-hot over 16)
            eqa = wp.tile([Pc, D * K2], f32)
            eqav = eqa[:].rearrange("p (d c) -> p d c", c=K2)
            nc.vector.tensor_tensor(
                eqav,
                nap_f[:].unsqueeze(2).broadcast_to([Pc, D, K2]),
                iotaK2_4.unsqueeze(1).broadcast_to([Pc, D, K2]),
                op=ALU.is_equal,
            )
            tmpa = wp.tile([Pc, D * K2], f32)
            tmpav = tmpa[:].rearrange("p (d c) -> p d c", c=K2)
            nc.vector.tensor_tensor(
                tmpav, eqav, beam_f[:].unsqueeze(1).broadcast_to([Pc, D, K2]),
                op=ALU.mult)
            att_f = wp.tile([Pc, D], f32)
            nc.vector.reduce_sum(att_f[:].unsqueeze(2), tmpav, axis=AX.X)
            nc.vector.tensor_tensor(
                tmpav, eqav, jtok[:].unsqueeze(1).broadcast_to([Pc, D, K2]),
                op=ALU.mult)
            atok_f = wp.tile([Pc, D], f32)
            nc.vector.reduce_sum(atok_f[:].unsqueeze(2), tmpav, axis=AX.X)

            spb8 = sp_u8[:, 0:1].broadcast_to([Pc, D])

            # new_alive_lp = where(sp, alive_lp, nav)
            nalp = wp.tile([Pc, D], f32)
            nc.vector.tensor_copy(nalp[:], nav[:])
            nc.vector.copy_predicated(nalp[:], spb8, alive8[:])
            nc.sync.dma_start(na_lp_out.ap(), nalp[:])

            # attention_change_ids = where(sp, arange(D), att_f)
            nc.vector.copy_predicated(att_f[:], spb8, iotaD_4)
            att_i = wp.tile([Pc, D], i32)
            nc.vector.tensor_copy(att_i[:], att_f[:])
            nc.sync.dma_start(att_out.ap(), att_i[:])

            # alive seq source rows: base + att (post-sp)
            arow_f = wp.tile([Pc, D], f32)
            nc.vector.tensor_add(arow_f[:], base8_f, att_f[:])

            # alive token-write flag = !sp
            wfa = wp.tile([Pc, D], f32)
            nc.vector.tensor_scalar(wfa[:], sp1[:, 0:1].broadcast_to([Pc, D]),
                                    -1.0, 1.0, op0=ALU.mult, op1=ALU.add)

            # ----- fin side -----
            notf = wp.tile([Pc, K2], f32)
            nc.vector.tensor_scalar(notf[:], fino[:], -1.0, 1.0, op0=ALU.mult, op1=ALU.add)
            fm = wp.tile([Pc, K2], f32)             # fin-masked = jv + (1-fino)*(-INF)
            nc.vector.tensor_scalar(fm[:], notf[:], -INF, None, op0=ALU.mult)
            nc.vector.tensor_add(fm[:], fm[:], jv[:])

            cat = wp.tile([Pc, D + K2], f32)        # [fin_lp(8) | fin_masked(16)]
            nc.vector.tensor_copy(cat[:, 0:D], fin8[:])
            nc.vector.tensor_copy(cat[:, D:D + K2], fm[:])

            nfv = wp.tile([Pc, D], f32)
            nfp = wp.tile([Pc, D], u32)
            nc.vector.max(out=nfv[:], in_=cat[:])
            nc.vector.max_index(out=nfp[:], in_max=nfv[:], in_values=cat[:])
            nfp_f = wp.tile([Pc, D], f32)
            nc.vector.tensor_copy(nfp_f[:], nfp[:])

            # new_fin_lp = where(sp, fin_lp, nfv)
            nflp = wp.tile([Pc, D], f32)
            nc.vector.tensor_copy(nflp[:], nfv[:])
            nc.vector.copy_predicated(nflp[:], spb8, fin8[:])
            nc.sync.dma_start(nf_lp_out.ap(), nflp[:])

            # old/new resolution
            isold = wp.tile([Pc, D], f32)           # nfp < 8 (or sp)
            nc.vector.tensor_scalar(isold[:], nfp_f[:], float(D), None, op0=ALU.is_lt)
            nc.vector.copy_predicated(isold[:], spb8, ones8)
            isold_u8 = wp.tile([Pc, D], u8)
            nc.vector.tensor_copy(isold_u8[:], isold[:])
            oldrow = wp.tile([Pc, D], f32)
            nc.vector.tensor_scalar(oldrow[:], nfp_f[:], float(D - 1), None, op0=ALU.min)
            nc.vector.copy_predicated(oldrow[:], spb8, iotaD_4)
            cand = wp.tile([Pc, D], f32)            # clamp(nfp-8, 0) -> topk entry id
            nc.vector.tensor_scalar(cand[:], nfp_f[:], float(D), 0.0,
                                    op0=ALU.subtract, op1=ALU.max)

            # gather beam/token for new-fin entries (one-hot over 16)
            eqf = wp.tile([Pc, D * K2], f32)
            eqfv = eqf[:].rearrange("p (d c) -> p d c", c=K2)
            nc.vector.tensor_tensor(
                eqfv,
                cand[:].unsqueeze(2).broadcast_to([Pc, D, K2]),
                iotaK2_4.unsqueeze(1).broadcast_to([Pc, D, K2]),
                op=ALU.is_equal,
            )
            nc.vector.tensor_tensor(
                tmpav, eqfv, beam_f[:].unsqueeze(1).broadcast_to([Pc, D, K2]),
                op=ALU.mult)
            fbeam_f = wp.tile([Pc, D], f32)
            nc.vector.reduce_sum(fbeam_f[:].unsqueeze(2), tmpav, axis=AX.X)
            nc.vector.tensor_tensor(
                tmpav, eqfv, jtok[:].unsqueeze(1).broadcast_to([Pc, D, K2]),
                op=ALU.mult)
            ftok_f = wp.tile([Pc, D], f32)
            nc.vector.reduce_sum(ftok_f[:].unsqueeze(2), tmpav, axis=AX.X)

            # fin source row in the combined seqs tensor:
            # old -> R + base + oldrow ; new -> base + fbeam
            frow_f = wp.tile([Pc, D], f32)
            nc.vector.tensor_add(frow_f[:], base8_f, fbeam_f[:])
            frow_old_f = wp.tile([Pc, D], f32)
            nc.vector.tensor_scalar(frow_old_f[:], oldrow[:], 1.0, float(R),
                                    op0=ALU.mult, op1=ALU.add)
            nc.vector.tensor_add(frow_old_f[:], frow_old_f[:], base8_f)
            nc.vector.copy_predicated(frow_f[:], isold_u8[:], frow_old_f[:])

            # fin token-write flag = !sp & !isold
            notold = wp.tile([Pc, D], f32)
            nc.vector.tensor_scalar(notold[:], isold[:], -1.0, 1.0, op0=ALU.mult, op1=ALU.add)
            wff = wp.tile([Pc, D], f32)
            nc.vector.tensor_tensor(wff[:], wfa[:], notold[:], op=ALU.mult)

            # pack per-row (32) controls, d-major: [wfa, wff, atok, ftok, arow, frow]
            NF = 6
            pack = wp.tile([Pc, D * NF], f32)
            pack_v = pack[:].rearrange("p (d f) -> p d f", f=NF)
            fields = [wfa, wff, atok_f, ftok_f, arow_f, frow_f]
            for f_i, fld in enumerate(fields):
                nc.vector.tensor_copy(pack_v[:, :, f_i:f_i + 1],
                                      fld[:].unsqueeze(2))
            unpk = wp.tile([R, NF], f32)
            nc.sync.dma_start(unpk[:], pack_v)
            atok_i = wp.tile([R, 1], i32)
            nc.vector.tensor_copy(atok_i[:], unpk[:, 2:3])
            ftok_i = wp.tile([R, 1], i32)
            nc.vector.tensor_copy(ftok_i[:], unpk[:, 3:4])
            arow_i = wp.tile([R, 1], i32)
            nc.vector.tensor_copy(arow_i[:], unpk[:, 4:5])
            frow_i = wp.tile([R, 1], i32)
            nc.vector.tensor_copy(frow_i[:], unpk[:, 5:6])

            wa_u8 = wp.tile([R, 1], u8)             # write_alive flag
            nc.vector.tensor_copy(wa_u8[:], unpk[:, 0:1])
            wf_u8 = wp.tile([R, 1], u8)             # write_fin flag
            nc.vector.tensor_copy(wf_u8[:], unpk[:, 1:2])

            # ---------------- seq gathers + token writes + outputs -----------
            stage_a = sgp.tile([R, S], i32)
            nc.gpsimd.indirect_dma_start(
                out=stage_a[:], out_offset=None, in_=seqs.ap(),
                in_offset=bass.IndirectOffsetOnAxis(ap=arow_i[:], axis=0))
            stage_f = sgp.tile([R, S], i32)
            nc.gpsimd.indirect_dma_start(
                out=stage_f[:], out_offset=None, in_=seqs.ap(),
                in_offset=bass.IndirectOffsetOnAxis(ap=frow_i[:], axis=0))

            # write new token at the cur_pos column of the staged rows
            # (cur_pos is specialized at compile time; recompiled if it changes)
            cp = int(cur_pos)
            nc.vector.copy_predicated(
                stage_a[:, cp:cp + 1], wa_u8[:], atok_i[:])
            nc.vector.copy_predicated(
                stage_f[:, cp:cp + 1], wf_u8[:], ftok_i[:])

            nc.sync.dma_start(na_seq_out.ap(), stage_a[:])
            nc.sync.dma_start(nf_seq_out.ap(), stage_f[:])

    return nc


_CACHED_NC = {}


def _get_nc(cur_pos=1024):
    cp = int(cur_pos)
    if cp not in _CACHED_NC:
        nc = bacc.Bacc("TRN2", target_bir_lowering=False, debug=False)
        build_core_program(nc, cp)
        nc.compile()
        _CACHED_NC[cp] = nc
    return _CACHED_NC[cp]


def make_in_maps(probs, alive_seq, fin_seq, alive_log_probs, fin_log_probs,
                 still_prompt, is_first, cur_pos):
    probs = np.asarray(probs, dtype=np.float32).reshape(P_FULL, D, V)
    alive_seq = np.asarray(alive_seq, dtype=np.int32)
    fin_seq = np.asarray(fin_seq, dtype=np.int32)
    alive_log_probs = np.asarray(alive_log_probs, dtype=np.float32)
    fin_log_probs = np.asarray(fin_log_probs, dtype=np.float32)
    still_prompt = np.asarray(still_prompt).astype(np.float32).reshape(P_FULL, 1)
    is_first = np.asarray(is_first).astype(np.float32).reshape(P_FULL, 1)
    cp = int(np.asarray(cur_pos))

    in_maps = []
    for c in range(N_CORES):
        s = slice(c * Pc, (c + 1) * Pc)
        seqs = np.concatenate([alive_seq[s].reshape(R, S),
                               fin_seq[s].reshape(R, S)], axis=0)
        in_maps.append({
            "probs": np.ascontiguousarray(probs[s].reshape(R, V)),
            "seqs": np.ascontiguousarray(seqs),
            "alive_lp": np.ascontiguousarray(alive_log_probs[s]),
            "fin_lp": np.ascontiguousarray(fin_log_probs[s]),
            "sp": np.ascontiguousarray(still_prompt[s]),
            "isf": np.ascontiguousarray(is_first[s]),
        })
    return in_maps


def assemble_outputs(results):
    att = np.concatenate([r["att"] for r in results], axis=0).astype(np.int32)
    na_seq = np.concatenate(
        [r["na_seq"].reshape(Pc, D, S) for r in results], axis=0).astype(np.int32)
    na_lp = np.concatenate([r["na_lp"] for r in results], axis=0).astype(np.float32)
    nf_seq = np.concatenate(
        [r["nf_seq"].reshape(Pc, D, S) for r in results], axis=0).astype(np.int32)
    nf_lp = np.concatenate([r["nf_lp"] for r in results], axis=0).astype(np.float32)
    return (att, na_seq, na_lp, nf_seq, nf_lp)


def kernel(probs, alive_seq, fin_seq, alive_log_probs, fin_log_probs,
           still_prompt, is_first, cur_pos, _trace=False, _trace_kwargs=None):
    from concourse.bass_utils import run_bass_kernel_spmd

    nc = _get_nc(cur_pos)
    in_maps = make_in_maps(probs, alive_seq, fin_seq, alive_log_probs,
                           fin_log_probs, still_prompt, is_first, cur_pos)
    res = run_bass_kernel_spmd(
        nc, in_maps, core_ids=list(range(N_CORES)), trace=_trace,
        **(_trace_kwargs or {}))
    out = assemble_outputs(res.results)
    if _trace:
        return out, res
    return out
